# revision 24
# baseline (speedup 1.0000x reference)
"""MLA (multi-head latent attention) Bass kernel for Trainium2, 8 NeuronCores.

Sharding: core i handles batch b = i // 2 and head-group g2 = i % 2
(8 of the 16 heads).  Each core computes a partial output
(its heads' contribution through out_proj); the host sums the two
partials per batch and adds b_o.

Design (ACT-bound; softmax exp on ScalarE is the per-core floor):
  - Host pre-lays-out everything: x transposed to bf16 xT [128,8,S] and
    fp8 x8T [64,8,2,S]; weights pre-cast (bf16 / fp8), K/Q up-projection
    columns pre-permuted so the fp8 DoubleRow layout falls out of plain
    PSUM evacuations.
  - QK^T runs in fp8e4 DoubleRow: KT8/QT8 stored [128p, g, plane, S]
    (partition 32a+p, plane pl = head 4g+a, dim 32pl+p); one matmul
    contracts all 64 head dims at 0.5 cycles/col.  The whole Q path
    (x->q_lat->QT) is fp8 DoubleRow too - it only feeds softmax scores,
    which tolerate fp8 noise.  V/out paths stay bf16.
  - Emission order software-pipelines: pieces 0-1 up front (deep scoped
    PSUM pool, KT/QT evacuations on the then-idle ACT engine), pieces
    2-3 as fillers inside j=0 attention, out-proj of the first token
    half as fillers inside j=1, remainder in a deep-pool tail with ACT
    evacuations.
  - PSUM: attention = scores [128,1024]x2bufs (4 banks) + ctx [65,1024]
    (2) + filler work tiles [128,512]x2 (2).
"""

import numpy as np
import ml_dtypes

import concourse.bass as bass
import concourse.bacc as bacc
import concourse.mybir as mybir
import concourse.tile as tile

DIM = 1024
NUM_HEADS = 16
HEAD_DIM = 64
LAT = 128
QR = 256
B = 4
NCORES = 8
ND = DIM // 128       # 8 d-chunks
NHL = 8               # heads per core
F32 = mybir.dt.float32
BF16 = mybir.dt.bfloat16
FP8 = mybir.dt.float8e4
AF = mybir.ActivationFunctionType
ALU = mybir.AluOpType
DR = mybir.MatmulPerfMode.DoubleRow


def _pieces(total, w=512):
    return [(o, min(w, total - o)) for o in range(0, total, w)]


def build_mla(S=2048):
    """Build the per-core Bass program (same SPMD program on all 8 cores)."""
    assert S % 1024 == 0
    SH = S // 2           # s-half width
    NT = S // 128         # number of 128-token chunks
    NP = S // 512         # number of 512-token pieces

    nc = bacc.Bacc()

    x_d = nc.declare_dram_parameter("x", [128, ND, S], BF16, isOutput=False)
    x8_d = nc.declare_dram_parameter("x8", [64, ND, 2, S], FP8, isOutput=False)
    w_kvc_d = nc.declare_dram_parameter("w_kvc", [128, ND, LAT], BF16, isOutput=False)
    w_qc8_d = nc.declare_dram_parameter("w_qc8", [64, ND, 2, QR], FP8, isOutput=False)
    w_kvu_k_d = nc.declare_dram_parameter("w_kvu_k", [128, 512], BF16, isOutput=False)
    w_qu8_d = nc.declare_dram_parameter("w_qu8", [128, 2, 512], FP8, isOutput=False)
    w_kvu_v_d = nc.declare_dram_parameter("w_kvu_v", [128, 512], BF16, isOutput=False)
    w_o_d = nc.declare_dram_parameter("w_o", [128, 4, DIM], BF16, isOutput=False)
    b_kvc_d = nc.declare_dram_parameter("b_kvc", [LAT, 1], F32, isOutput=False)
    b_qc_d = nc.declare_dram_parameter("b_qc", [128, 2], F32, isOutput=False)
    b_qu_d = nc.declare_dram_parameter("b_qu", [128, 4], F32, isOutput=False)
    b_kvu_k_d = nc.declare_dram_parameter("b_kvu_k", [128, 4], F32, isOutput=False)
    b_kvu_v_d = nc.declare_dram_parameter("b_kvu_v", [1, 512], F32, isOutput=False)
    out_d = nc.declare_dram_parameter("out", [S, DIM], F32, isOutput=True)

    with tile.TileContext(nc) as tc:
        with (
            tc.tile_pool(name="wts", bufs=1) as wts,
            tc.tile_pool(name="big", bufs=1) as big,
            tc.tile_pool(name="lat", bufs=2) as latp,
            tc.tile_pool(name="exb", bufs=4) as exb,
            tc.tile_pool(name="nrm", bufs=2) as nrm,
            tc.tile_pool(name="obp", bufs=4) as obp,
        ):
            # ---- early ACT-queue DMAs: weights for the first matmuls
            # lead, then biases (needed only at evac time); a dummy exp
            # preloads the activation table off the critical path --------
            w_kvc_sb = wts.tile([128, ND, LAT], BF16, name="w_kvc_sb")
            nc.scalar.dma_start(out=w_kvc_sb[:], in_=w_kvc_d[:, :, :])
            atl = wts.tile([1, 1], F32, name="atl")
            nc.gpsimd.memset(atl[:], 0.0)
            nc.scalar.activation(atl[:], atl[:], AF.Exp, scale=1.0)
            w_qc8_sb = wts.tile([64, ND, 2, QR], FP8, name="w_qc8_sb")
            nc.scalar.dma_start(out=w_qc8_sb[:], in_=w_qc8_d[:, :, :, :])
            w_qu8_sb = wts.tile([128, 2, 512], FP8, name="w_qu8_sb")
            nc.scalar.dma_start(out=w_qu8_sb[:], in_=w_qu8_d[:, :, :])
            w_kvu_k_sb = wts.tile([128, 512], BF16, name="w_kvu_k_sb")
            nc.scalar.dma_start(out=w_kvu_k_sb[:], in_=w_kvu_k_d[:, :])
            b_kvc_sb = wts.tile([128, 1], F32, name="b_kvc_sb")
            nc.scalar.dma_start(out=b_kvc_sb[:], in_=b_kvc_d[:, :])
            b_qc_sb = wts.tile([128, 2], F32, name="b_qc_sb")
            nc.scalar.dma_start(out=b_qc_sb[:], in_=b_qc_d[:, :])
            b_qu_sb = wts.tile([128, 4], F32, name="b_qu_sb")
            nc.scalar.dma_start(out=b_qu_sb[:], in_=b_qu_d[:, :])
            b_kvu_k_sb = wts.tile([128, 4], F32, name="b_kvu_k_sb")
            nc.scalar.dma_start(out=b_kvu_k_sb[:], in_=b_kvu_k_d[:, :])
            bv_row = wts.tile([1, 512], F32, name="bv_row")
            nc.scalar.dma_start(out=bv_row[:], in_=b_kvu_v_d[:, :])
            bvb = wts.tile([128, 512], F32, name="bvb")
            nc.gpsimd.partition_broadcast(bvb[:], bv_row[0:1, :])
            w_kvu_v_sb = wts.tile([128, 512], BF16, name="w_kvu_v_sb")
            nc.scalar.dma_start(out=w_kvu_v_sb[:], in_=w_kvu_v_d[:, :])

            # identity (bf16) for PSUM re-injection in the out-proj tail
            from concourse import masks
            identf = wts.tile([128, 128], F32, name="identf")
            masks.make_identity(nc, identf[:])
            ident16 = wts.tile([128, 128], BF16, name="ident16")
            nc.gpsimd.tensor_copy(ident16[:], identf[:])

            # ---- xT / x8T on the SP queue, piece-major --------------------
            xT = big.tile([128, ND, S], BF16, name="xT")
            x8T = big.tile([64, ND, 2, S], FP8, name="x8T")
            for p in range(NP):
                nc.sync.dma_start(
                    out=xT[:, :, 512 * p:512 * p + 512],
                    in_=x_d[:, :, 512 * p:512 * p + 512])
                nc.sync.dma_start(
                    out=x8T[:, :, :, 512 * p:512 * p + 512],
                    in_=x8_d[:, :, :, 512 * p:512 * p + 512])

            # w_o rides the SP queue after xT/x8 (needed only in phase E)
            w_o_sb = wts.tile([128, 4, DIM], BF16, name="w_o_sb")
            nc.sync.dma_start(out=w_o_sb[:], in_=w_o_d[:, :, :])

            # ---- persistent tensors ---------------------------------------
            # KT8/QT8: [128p, g, plane, S]; partition 32a+p, plane pl
            # holds head 4g+a, dim 32*pl+p (fp8 for DoubleRow QK).
            KT8 = big.tile([128, 2, 2, S], FP8, name="KT8")
            QT8 = big.tile([128, 2, 2, S], FP8, name="QT8")
            # V: [128tok, chunk, head, 65] (64 vals + ones col)
            V = big.tile([128, NT, NHL, 65], BF16, name="V")
            nc.gpsimd.memset(V[:, :, :, 64:65], 1.0)
            # ctxT: [128 (2 heads x 64 dims), chunk h//2, S]
            ctxT = big.tile([128, 4, S], BF16, name="ctxT")

            # ---- work-unit emitters (pool + evac engine parameterized) ----
            def evac(on_act, dst, src, bias):
                if on_act:
                    nc.scalar.activation(dst, src, AF.Identity, bias=bias)
                else:
                    nc.vector.tensor_scalar_add(dst, src, bias)

            def unit_kv(pool, p):
                off = 512 * p
                kvp = pool.tile([128, 512], F32, tag="wk")
                for dc in range(ND):
                    nc.tensor.matmul(
                        kvp[:], w_kvc_sb[:, dc, :],
                        xT[:, dc, off:off + 512],
                        start=(dc == 0), stop=(dc == ND - 1))
                kvs = latp.tile([128, 512], BF16, tag="kvs")
                nc.vector.tensor_scalar_add(kvs[:], kvp[:], b_kvc_sb[:, 0:1])
                return kvs

            def unit_q(pool, p, qh, q8):
                off = 512 * p
                qp = pool.tile([128, 512], F32, tag="wk")
                for o in (0, 256):
                    for dc in range(ND):
                        nc.tensor.matmul(
                            qp[:, o:o + 256],
                            w_qc8_sb[:, dc, :, 128 * qh:128 * qh + 128],
                            x8T[:, dc, :, off + o:off + o + 256],
                            start=(dc == 0), stop=(dc == ND - 1),
                            perf_mode=DR)
                nc.vector.tensor_scalar_add(q8[:, qh, :], qp[:],
                                            b_qc_sb[:, qh:qh + 1])

            def unit_KT(pool, p, j, kvs, on_act=False):
                off = 512 * p
                kp = pool.tile([128, 512], F32, tag="wk")
                nc.tensor.matmul(kp[:], w_kvu_k_sb[:, 128 * j:128 * j + 128],
                                 kvs[:], start=True, stop=True)
                evac(on_act, KT8[:, j // 2, j % 2, off:off + 512], kp[:],
                     b_kvu_k_sb[:, j:j + 1])

            def unit_QT(pool, p, j, q8, on_act=False):
                off = 512 * p
                qp = pool.tile([128, 512], F32, tag="wk")
                for o in (0, 256):
                    nc.tensor.matmul(
                        qp[:, o:o + 256], w_qu8_sb[:, :, 128 * j:128 * j + 128],
                        q8[:, :, o:o + 256],
                        start=True, stop=True, perf_mode=DR)
                evac(on_act, QT8[:, j // 2, j % 2, off:off + 512], qp[:],
                     b_qu_sb[:, j:j + 1])

            def unit_V(pool, p, q, kvs):
                k = 4 * p + q
                vp = pool.tile([128, 512], F32, tag="wk")
                nc.tensor.matmul(vp[:], kvs[:, 128 * q:128 * q + 128],
                                 w_kvu_v_sb[:], start=True, stop=True)
                nc.vector.tensor_tensor(
                    V[:, k, :, 0:64],
                    vp[:].rearrange("p (h c) -> p h c", c=64),
                    bvb[:].rearrange("p (h c) -> p h c", c=64), ALU.add)

            def piece_units(pool, p, on_act=False, only=None):
                state = {}

                def mk_kv():
                    state["kvs"] = unit_kv(pool, p)

                def mk_q8():
                    q8 = latp.tile([128, 2, 512], FP8, tag="q8")
                    state["q8"] = q8
                    unit_q(pool, p, 0, q8)
                units = [("kv", mk_kv), ("q", mk_q8),
                         ("q", lambda: unit_q(pool, p, 1, state["q8"]))]
                units += [("KT", (lambda j=j: unit_KT(pool, p, j,
                                                      state["kvs"])))
                          for j in range(4)]
                units += [("QT", (lambda j=j: unit_QT(pool, p, j, state["q8"],
                                                      on_act)))
                          for j in range(4)]
                units += [("V", (lambda q=q: unit_V(pool, p, q, state["kvs"])))
                          for q in range(4)]
                for kind, u in units:
                    if only is None or kind in only:
                        yield u

            def unit_E(pool, si, o, dma_act=False):
                op = pool.tile([128, 512], F32, tag="wk")
                for cc in range(4):
                    nc.tensor.matmul(
                        op[:], ctxT[:, cc, 128 * si:128 * si + 128],
                        w_o_sb[:, cc, 512 * o:512 * o + 512],
                        start=(cc == 0), stop=(cc == 3))
                ob = obp.tile([128, 512], F32, tag="ob")
                nc.vector.tensor_copy(ob[:], op[:])
                eng = nc.scalar if dma_act else nc.sync
                eng.dma_start(
                    out=out_d[128 * si:128 * si + 128, 512 * o:512 * o + 512],
                    in_=ob[:])

            def unit_E1(pool, par, si, o):
                """Out-proj partial over cc 0,1 -> bf16 staging tile."""
                op = pool.tile([128, 512], F32, tag="wk")
                for cc in range(2):
                    nc.tensor.matmul(
                        op[:], ctxT[:, cc, 128 * si:128 * si + 128],
                        w_o_sb[:, cc, 512 * o:512 * o + 512],
                        start=(cc == 0), stop=(cc == 1))
                dst = par[:, 2 * (si - NT // 2) + o, :]
                nc.vector.tensor_copy(dst, op[:])

            def unit_E2(pool, par, si, o, alt=False):
                """cc2+cc3 matmuls, staged partial re-injected via identity
                matmul; plain-copy evac + DMA alternate between engines."""
                op = pool.tile([128, 512], F32, tag="wk")
                for cc in (2, 3):
                    nc.tensor.matmul(
                        op[:], ctxT[:, cc, 128 * si:128 * si + 128],
                        w_o_sb[:, cc, 512 * o:512 * o + 512],
                        start=(cc == 2), stop=False)
                nc.tensor.matmul(
                    op[:], ident16[:], par[:, 2 * (si - NT // 2) + o, :],
                    start=False, stop=True)
                ob = obp.tile([128, 512], F32, tag="ob")
                if alt:
                    nc.scalar.activation(ob[:], op[:], AF.Identity, bias=0.0)
                else:
                    nc.vector.tensor_copy(ob[:], op[:])
                eng = nc.scalar if alt else nc.sync
                eng.dma_start(
                    out=out_d[128 * si:128 * si + 128, 512 * o:512 * o + 512],
                    in_=ob[:])

            class Filler:
                """Dispenses queued work units evenly over `slots` calls."""
                def __init__(self, units, slots):
                    self.units = list(units)
                    self.slots = max(1, slots)
                    self.acc = 0.0
                    self.rate = len(self.units) / self.slots

                def __call__(self):
                    self.acc += self.rate
                    while self.acc >= 1.0 and self.units:
                        self.units.pop(0)()
                        self.acc -= 1.0

                def drain(self):
                    while self.units:
                        self.units.pop(0)()

            # ---- pieces 0..NP/2-1: deep scoped PSUM pool, ACT evacs -------
            with tc.tile_pool(name="pwk0", bufs=4, space="PSUM") as pwk0:
                # warm-up matmuls ramp the PE p-state while DMAs land
                warm = pwk0.tile([128, 512], F32, tag="warm")
                for i in range(8):
                    nc.tensor.matmul(warm[:, 0:128], ident16[:],
                                     ident16[:, 0:128],
                                     start=(i == 0), stop=(i == 7))
                for p in range(NP // 2):
                    for u in piece_units(pwk0, p, on_act=True):
                        u()


            def attn_phase(j, heads, filler, psc, pctx):
                """Attention for s-half j over `heads`, emitted with QK one
                chunk ahead of PV so exp never waits at head boundaries."""
                s0 = SH * j
                kmax = (SH // 128) * (j + 1)
                nbank = SH // 512
                last_k = {
                    bi: min(kmax - 1, (s0 + 512 * (bi + 1)) // 128 - 1)
                    for bi in range(nbank)
                }
                # chunk packing: complementary fd values share one score
                # tile (and one exp instruction) to amortize the ACT access
                # bubble.  Packs are emitted in order; PSUM accumulation
                # start/stop flags follow emission order per bank.
                if kmax == 8:
                    packs = [[0], [1, 7], [2, 6], [3, 5], [4]]
                else:
                    packs = [[k] for k in range(9)] + [[9, 15], [10, 14],
                                                       [11, 13], [12]]
                # emission order of chunks, for per-bank start/stop flags
                order = [k for pk in packs for k in pk]

                def chunk_geom(k):
                    t0 = 128 * k
                    ss = max(s0, t0)
                    return t0, ss, s0 + SH - ss, ss - s0

                bank_first, bank_last = {}, {}
                for pos, k in enumerate(order):
                    _, _, fd, rel = chunk_geom(k)
                    for bi in range(nbank):
                        a2 = max(rel, 512 * bi)
                        b2 = min(SH, 512 * bi + 512)
                        if a2 >= b2:
                            continue
                        if bi not in bank_first:
                            bank_first[bi] = k
                        bank_last[bi] = k

                recs = []
                for h in heads:
                    g, a = h // 4, h % 4
                    hst = {}
                    for pi, pk in enumerate(packs):
                        rec = {}
                        geo = [chunk_geom(k) for k in pk]
                        cos = []
                        co = 0
                        for (_, _, fd, _) in geo:
                            cos.append(co)
                            co += fd

                        def qk(rec=rec, g=g, a=a, pk=pk, geo=geo, cos=cos):
                            sc = psc.tile([128, SH], F32, tag="sc")
                            rec["sc"] = sc  # noqa
                            for k, (t0, ss, fd, rel), co in zip(pk, geo, cos):
                                for o2, w2 in _pieces(fd, 256):
                                    nc.tensor.matmul(
                                        sc[:, co + o2:co + o2 + w2],
                                        KT8[32 * a:32 * a + 32, g, :,
                                            t0:t0 + 128],
                                        QT8[32 * a:32 * a + 32, g, :,
                                            ss + o2:ss + o2 + w2],
                                        start=True, stop=True, perf_mode=DR,
                                        tile_position=(32 * a, 0))

                        def expaff(rec=rec, pk=pk, geo=geo, cos=cos):
                            ex = exb.tile([128, SH], BF16, tag="ex")
                            rec["ex"] = ex  # noqa
                            w = cos[-1] + geo[-1][2]
                            nc.scalar.activation(ex[:, :w], rec["sc"][:, :w],
                                                 AF.Exp, scale=0.125)
                            for k, (t0, ss, fd, rel), co in zip(pk, geo, cos):
                                if t0 >= s0:
                                    nc.gpsimd.affine_select(
                                        out=ex[:, co:co + 128],
                                        in_=ex[:, co:co + 128],
                                        pattern=[[1, 128]],
                                        compare_op=ALU.is_ge,
                                        fill=0.0, base=0,
                                        channel_multiplier=-1)

                        def pv(rec=rec, hst=hst, h=h, pk=pk, geo=geo, cos=cos):
                            if pk[0] == 0:
                                ctx = pctx.tile([65, SH], F32, tag="ctx")
                                hst["ctx"] = ctx
                            for k, (t0, ss, fd, rel), co in zip(pk, geo, cos):
                                for bi in range(nbank):
                                    a2 = max(rel, 512 * bi)
                                    b2 = min(SH, 512 * bi + 512)
                                    if a2 >= b2:
                                        continue
                                    nc.tensor.matmul(
                                        hst["ctx"][:, a2:b2], V[:, k, h, :],
                                        rec["ex"][:, co + a2 - rel:
                                                  co + b2 - rel],
                                        start=(bank_first[bi] == k),
                                        stop=(bank_last[bi] == k))

                        rec.update(qk=qk, expaff=expaff, pv=pv)
                        if pi == len(packs) - 1:
                            def norm(hst=hst, h=h):
                                # copy ctx out + reciprocal on DVE (releases
                                # the PSUM ctx buffer early); broadcast and
                                # multiply ride the lightly-loaded Pool engine
                                ctx = hst["ctx"]
                                raw = nrm.tile([64, SH], F32, tag="raw")
                                nc.vector.tensor_copy(raw[:], ctx[0:64, :])
                                rc = nrm.tile([1, SH], F32, tag="rec")
                                nc.vector.reciprocal(rc[:], ctx[64:65, :])
                                rbc = nrm.tile([64, SH], F32, tag="rbc")
                                nc.gpsimd.partition_broadcast(rbc[:],
                                                              rc[0:1, :])
                                po = 64 * (h % 2)
                                nc.gpsimd.tensor_tensor(
                                    ctxT[po:po + 64, h // 2, s0:s0 + SH],
                                    raw[:], rbc[:], ALU.mult)
                            rec["norm"] = norm
                        recs.append(rec)
                recs[0]["qk"]()
                for i, rec in enumerate(recs):
                    rec["expaff"]()
                    if i + 1 < len(recs):
                        recs[i + 1]["qk"]()
                    rec["pv"]()
                    if "norm" in rec:
                        rec["norm"]()
                    filler()

            # ---- attention (+ pieces 2-3 and first-half out-proj fillers) -
            with (
                tc.tile_pool(name="psc", bufs=2, space="PSUM") as psc,
                tc.tile_pool(name="pctx", bufs=1, space="PSUM") as pctx,
                tc.tile_pool(name="pwk", bufs=2, space="PSUM") as pwk,
            ):
                p2, p3 = NP // 2, NP // 2 + 1
                units_j0 = []
                st = {}
                for p in (p2, p3):
                    def mk_kv(p=p):
                        st[f"kvs{p}"] = unit_kv(pwk, p)
                    def mk_q8(p=p):
                        q8 = latp.tile([128, 2, 512], FP8, tag="q8")
                        st[f"q8{p}"] = q8
                        unit_q(pwk, p, 0, q8)
                    units_j0.append(mk_kv)
                    units_j0.append(mk_q8)
                    units_j0.append(lambda p=p: unit_q(pwk, p, 1, st[f"q8{p}"]))
                units_j0 += [(lambda p=p, j=j: unit_QT(pwk, p, j, st[f"q8{p}"]))
                             for p in (p2, p3) for j in range(4)]
                f0 = Filler(units_j0, 48)
                attn_phase(0, range(NHL), f0, psc, pctx)
                f0.drain()

                # j1 fillers: burst of late-consumed evacs (first consumer is
                # chunk >= 8 of head 0), then first-half out-proj
                burst = [(lambda p=p, j=j: unit_KT(pwk, p, j, st[f"kvs{p}"]))
                         for p in (p2, p3) for j in range(4)]
                burst += [(lambda p=p, q=q: unit_V(pwk, p, q, st[f"kvs{p}"]))
                          for p in (p2, p3) for q in range(4)]
                units_E = [
                    (lambda si=si, o=o: unit_E(pwk, si, o))
                    for si in range(NT // 2) for o in range(2)
                ]
                par = wts.tile([128, NT, 512], BF16, name="par")
                units_E1 = [
                    (lambda si=si, o=o: unit_E1(pwk, par, si, o))
                    for si in range(NT // 2, NT) for o in range(2)
                ]
                pk_per_head = 13         # packs per head in j1
                fb = Filler(burst, 7)
                fe = Filler(units_E, 4 * pk_per_head - 7)
                f1e = Filler(units_E1, 4 * pk_per_head)
                slot = [0]

                def f1():
                    slot[0] += 1
                    if fb.units:
                        fb()
                    elif fe.units:
                        fe()
                    elif slot[0] > 4 * pk_per_head:
                        f1e()
                attn_phase(1, range(NHL), f1, psc, pctx)
                fb.drain()
                fe.drain()
                f1e.drain()

            # ---- tail: remaining out-proj with a deep pool, ACT evacs -----
            with tc.tile_pool(name="ptl", bufs=4, space="PSUM") as ptl:
                for i, si in enumerate(range(NT // 2, NT)):
                    for o in range(2):
                        unit_E2(ptl, par, si, o, alt=(o == 1))

    nc.finalize()
    return nc


def _perm512():
    """Column permutation for w_kvu_k / w_qu so that PSUM chunk j, row
    32a+p corresponds to head 4*(j//2)+a, dim 32*(j%2)+p."""
    perm = np.empty(512, dtype=np.int64)
    for j in range(4):
        for a in range(4):
            for p in range(32):
                perm[128 * j + 32 * a + p] = 64 * (4 * (j // 2) + a) + 32 * (j % 2) + p
    return perm


def shard_inputs(inputs, S=2048):
    """Build the 8 per-core input maps from full inputs (host-side prep)."""
    f32 = lambda a: np.ascontiguousarray(np.asarray(a, dtype=np.float32))
    bf = lambda a: np.ascontiguousarray(
        np.asarray(a, dtype=np.float32).astype(ml_dtypes.bfloat16))
    fp8 = lambda a: np.ascontiguousarray(
        np.asarray(a, dtype=np.float32).astype(ml_dtypes.float8_e4m3))
    x = f32(inputs["x"])
    w_kvc, b_kvc = f32(inputs["w_kvc"]), f32(inputs["b_kvc"])
    w_kvu, b_kvu = f32(inputs["w_kvu"]), f32(inputs["b_kvu"])
    w_qc, b_qc = f32(inputs["w_qc"]), f32(inputs["b_qc"])
    w_qu, b_qu = f32(inputs["w_qu"]), f32(inputs["b_qu"])
    w_o, b_o = f32(inputs["w_o"]), f32(inputs["b_o"])
    perm = _perm512()
    in_maps = []
    for core in range(NCORES):
        b = core // 2
        g2 = core % 2
        ks = slice(512 * g2, 512 * g2 + 512)            # K-feature slice
        vs = slice(DIM + 512 * g2, DIM + 512 * g2 + 512)  # V-feature slice
        in_maps.append({
            "x": bf(x[b].T.reshape(ND, 128, S).transpose(1, 0, 2)),
            "x8": fp8(x[b].T.reshape(ND, 2, 64, S).transpose(2, 0, 1, 3)),
            "w_kvc": bf(w_kvc.reshape(ND, 128, LAT).transpose(1, 0, 2)),
            "w_qc8": fp8(w_qc.reshape(ND, 2, 64, QR).transpose(2, 0, 1, 3)),
            "w_kvu_k": bf(w_kvu[:, ks][:, perm]),
            "w_qu8": fp8(w_qu[:, ks][:, perm].reshape(2, 128, 512).transpose(1, 0, 2)),
            "w_kvu_v": bf(w_kvu[:, vs]),
            "w_o": bf(w_o[ks, :].reshape(4, 128, DIM).transpose(1, 0, 2)),
            "b_kvc": f32(b_kvc.reshape(LAT, 1)),
            "b_qc": f32(b_qc.reshape(2, 128).T),
            "b_qu": f32(b_qu[ks][perm].reshape(4, 128).T),
            "b_kvu_k": f32(b_kvu[ks][perm].reshape(4, 128).T),
            "b_kvu_v": f32(b_kvu[vs].reshape(1, 512)),
        })
    return in_maps


def kernel(**inputs) -> np.ndarray:
    from concourse.bass_utils import run_bass_kernel_spmd

    x = np.asarray(inputs["x"])
    S = x.shape[1]
    nc = build_mla(S=S)
    in_maps = shard_inputs(inputs, S=S)
    res = run_bass_kernel_spmd(nc, in_maps, list(range(NCORES))).results
    b_o = np.asarray(inputs["b_o"], dtype=np.float32)
    out = np.empty((B, S, DIM), dtype=np.float32)
    for b in range(B):
        out[b] = res[2 * b]["out"] + res[2 * b + 1]["out"] + b_o
    return out


# revision 25
# speedup vs baseline: 1.0073x; 1.0073x over previous
"""MLA (multi-head latent attention) Bass kernel for Trainium2, 8 NeuronCores.

Sharding: core i handles batch b = i // 2 and head-group g2 = i % 2
(8 of the 16 heads).  Each core computes a partial output
(its heads' contribution through out_proj); the host sums the two
partials per batch and adds b_o.

Design (ACT-bound; softmax exp on ScalarE is the per-core floor):
  - Host pre-lays-out everything: x transposed to bf16 xT [128,8,S] and
    fp8 x8T [64,8,2,S]; weights pre-cast (bf16 / fp8), K/Q up-projection
    columns pre-permuted so the fp8 DoubleRow layout falls out of plain
    PSUM evacuations.
  - QK^T runs in fp8e4 DoubleRow: KT8/QT8 stored [128p, g, plane, S]
    (partition 32a+p, plane pl = head 4g+a, dim 32pl+p); one matmul
    contracts all 64 head dims at 0.5 cycles/col.  The whole Q path
    (x->q_lat->QT) is fp8 DoubleRow too - it only feeds softmax scores,
    which tolerate fp8 noise.  V/out paths stay bf16.
  - Emission order software-pipelines: pieces 0-1 up front (deep scoped
    PSUM pool, KT/QT evacuations on the then-idle ACT engine), pieces
    2-3 as fillers inside j=0 attention, out-proj of the first token
    half as fillers inside j=1, remainder in a deep-pool tail with ACT
    evacuations.
  - PSUM: attention = scores [128,1024]x2bufs (4 banks) + ctx [65,1024]
    (2) + filler work tiles [128,512]x2 (2).
"""

import numpy as np
import ml_dtypes

import concourse.bass as bass
import concourse.bacc as bacc
import concourse.mybir as mybir
import concourse.tile as tile

DIM = 1024
NUM_HEADS = 16
HEAD_DIM = 64
LAT = 128
QR = 256
B = 4
NCORES = 8
ND = DIM // 128       # 8 d-chunks
NHL = 8               # heads per core
F32 = mybir.dt.float32
BF16 = mybir.dt.bfloat16
FP8 = mybir.dt.float8e4
AF = mybir.ActivationFunctionType
ALU = mybir.AluOpType
DR = mybir.MatmulPerfMode.DoubleRow


def _pieces(total, w=512):
    return [(o, min(w, total - o)) for o in range(0, total, w)]


def build_mla(S=2048):
    """Build the per-core Bass program (same SPMD program on all 8 cores)."""
    assert S % 1024 == 0
    SH = S // 2           # s-half width
    NT = S // 128         # number of 128-token chunks
    NP = S // 512         # number of 512-token pieces

    nc = bacc.Bacc()

    x_d = nc.declare_dram_parameter("x", [128, ND, S], BF16, isOutput=False)
    x8_d = nc.declare_dram_parameter("x8", [64, ND, 2, S], FP8, isOutput=False)
    w_kvc_d = nc.declare_dram_parameter("w_kvc", [128, ND, LAT], BF16, isOutput=False)
    w_qc8_d = nc.declare_dram_parameter("w_qc8", [64, ND, 2, QR], FP8, isOutput=False)
    w_kvu_k_d = nc.declare_dram_parameter("w_kvu_k", [128, 512], BF16, isOutput=False)
    w_qu8_d = nc.declare_dram_parameter("w_qu8", [128, 2, 512], FP8, isOutput=False)
    w_kvu_v_d = nc.declare_dram_parameter("w_kvu_v", [128, 512], BF16, isOutput=False)
    w_o_d = nc.declare_dram_parameter("w_o", [128, 4, DIM], BF16, isOutput=False)
    b_kvc_d = nc.declare_dram_parameter("b_kvc", [LAT, 1], F32, isOutput=False)
    b_qc_d = nc.declare_dram_parameter("b_qc", [128, 2], F32, isOutput=False)
    b_qu_d = nc.declare_dram_parameter("b_qu", [128, 4], F32, isOutput=False)
    b_kvu_k_d = nc.declare_dram_parameter("b_kvu_k", [128, 4], F32, isOutput=False)
    b_kvu_v_d = nc.declare_dram_parameter("b_kvu_v", [1, 512], F32, isOutput=False)
    out_d = nc.declare_dram_parameter("out", [S, DIM], F32, isOutput=True)

    with tile.TileContext(nc) as tc:
        with (
            tc.tile_pool(name="wts", bufs=1) as wts,
            tc.tile_pool(name="big", bufs=1) as big,
            tc.tile_pool(name="lat", bufs=2) as latp,
            tc.tile_pool(name="exb", bufs=4) as exb,
            tc.tile_pool(name="nrm", bufs=2) as nrm,
            tc.tile_pool(name="obp", bufs=4) as obp,
        ):
            # ---- early ACT-queue DMAs: weights for the first matmuls
            # lead, then biases (needed only at evac time); a dummy exp
            # preloads the activation table off the critical path --------
            w_kvc_sb = wts.tile([128, ND, LAT], BF16, name="w_kvc_sb")
            nc.scalar.dma_start(out=w_kvc_sb[:], in_=w_kvc_d[:, :, :])
            atl = wts.tile([1, 1], F32, name="atl")
            nc.gpsimd.memset(atl[:], 0.0)
            nc.scalar.activation(atl[:], atl[:], AF.Exp, scale=1.0)
            w_qc8_sb = wts.tile([64, ND, 2, QR], FP8, name="w_qc8_sb")
            nc.scalar.dma_start(out=w_qc8_sb[:], in_=w_qc8_d[:, :, :, :])
            w_qu8_sb = wts.tile([128, 2, 512], FP8, name="w_qu8_sb")
            nc.scalar.dma_start(out=w_qu8_sb[:], in_=w_qu8_d[:, :, :])
            w_kvu_k_sb = wts.tile([128, 512], BF16, name="w_kvu_k_sb")
            nc.scalar.dma_start(out=w_kvu_k_sb[:], in_=w_kvu_k_d[:, :])
            b_kvc_sb = wts.tile([128, 1], F32, name="b_kvc_sb")
            nc.scalar.dma_start(out=b_kvc_sb[:], in_=b_kvc_d[:, :])
            b_qc_sb = wts.tile([128, 2], F32, name="b_qc_sb")
            nc.scalar.dma_start(out=b_qc_sb[:], in_=b_qc_d[:, :])
            b_qu_sb = wts.tile([128, 4], F32, name="b_qu_sb")
            nc.scalar.dma_start(out=b_qu_sb[:], in_=b_qu_d[:, :])
            b_kvu_k_sb = wts.tile([128, 4], F32, name="b_kvu_k_sb")
            nc.scalar.dma_start(out=b_kvu_k_sb[:], in_=b_kvu_k_d[:, :])
            bv_row = wts.tile([1, 512], F32, name="bv_row")
            nc.scalar.dma_start(out=bv_row[:], in_=b_kvu_v_d[:, :])
            bvb = wts.tile([128, 512], F32, name="bvb")
            nc.gpsimd.partition_broadcast(bvb[:], bv_row[0:1, :])
            w_kvu_v_sb = wts.tile([128, 512], BF16, name="w_kvu_v_sb")
            nc.scalar.dma_start(out=w_kvu_v_sb[:], in_=w_kvu_v_d[:, :])

            # identity (bf16) for PSUM re-injection in the out-proj tail
            from concourse import masks
            identf = wts.tile([128, 128], F32, name="identf")
            masks.make_identity(nc, identf[:])
            ident16 = wts.tile([128, 128], BF16, name="ident16")
            nc.gpsimd.tensor_copy(ident16[:], identf[:])

            # ---- xT / x8T on the SP queue, piece-major --------------------
            xT = big.tile([128, ND, S], BF16, name="xT")
            x8T = big.tile([64, ND, 2, S], FP8, name="x8T")
            for p in range(NP):
                nc.sync.dma_start(
                    out=xT[:, :, 512 * p:512 * p + 512],
                    in_=x_d[:, :, 512 * p:512 * p + 512])
                nc.sync.dma_start(
                    out=x8T[:, :, :, 512 * p:512 * p + 512],
                    in_=x8_d[:, :, :, 512 * p:512 * p + 512])

            # w_o rides the SP queue after xT/x8 (needed only in phase E)
            w_o_sb = wts.tile([128, 4, DIM], BF16, name="w_o_sb")
            nc.sync.dma_start(out=w_o_sb[:], in_=w_o_d[:, :, :])

            # ---- persistent tensors ---------------------------------------
            # KT8/QT8: [128p, g, plane, S]; partition 32a+p, plane pl
            # holds head 4g+a, dim 32*pl+p (fp8 for DoubleRow QK).
            KT8 = big.tile([128, 2, 2, S], FP8, name="KT8")
            QT8 = big.tile([128, 2, 2, S], FP8, name="QT8")
            # V: [128tok, chunk, head, 65] (64 vals + ones col)
            V = big.tile([128, NT, NHL, 65], BF16, name="V")
            nc.gpsimd.memset(V[:, :, :, 64:65], 1.0)
            # ctxT: [128 (2 heads x 64 dims), chunk h//2, S]
            ctxT = big.tile([128, 4, S], BF16, name="ctxT")

            # ---- work-unit emitters (pool + evac engine parameterized) ----
            def evac(on_act, dst, src, bias):
                if on_act:
                    nc.scalar.activation(dst, src, AF.Identity, bias=bias)
                else:
                    nc.vector.tensor_scalar_add(dst, src, bias)

            def unit_kv(pool, p):
                off = 512 * p
                kvp = pool.tile([128, 512], F32, tag="wk")
                for dc in range(ND):
                    nc.tensor.matmul(
                        kvp[:], w_kvc_sb[:, dc, :],
                        xT[:, dc, off:off + 512],
                        start=(dc == 0), stop=(dc == ND - 1))
                kvs = latp.tile([128, 512], BF16, tag="kvs")
                nc.vector.tensor_scalar_add(kvs[:], kvp[:], b_kvc_sb[:, 0:1])
                return kvs

            def unit_q(pool, p, qh, q8):
                off = 512 * p
                qp = pool.tile([128, 512], F32, tag="wk")
                for o in (0, 256):
                    for dc in range(ND):
                        nc.tensor.matmul(
                            qp[:, o:o + 256],
                            w_qc8_sb[:, dc, :, 128 * qh:128 * qh + 128],
                            x8T[:, dc, :, off + o:off + o + 256],
                            start=(dc == 0), stop=(dc == ND - 1),
                            perf_mode=DR)
                nc.vector.tensor_scalar_add(q8[:, qh, :], qp[:],
                                            b_qc_sb[:, qh:qh + 1])

            def unit_KT(pool, p, j, kvs, on_act=False):
                off = 512 * p
                kp = pool.tile([128, 512], F32, tag="wk")
                nc.tensor.matmul(kp[:], w_kvu_k_sb[:, 128 * j:128 * j + 128],
                                 kvs[:], start=True, stop=True)
                evac(on_act, KT8[:, j // 2, j % 2, off:off + 512], kp[:],
                     b_kvu_k_sb[:, j:j + 1])

            def unit_QT(pool, p, j, q8, on_act=False):
                off = 512 * p
                qp = pool.tile([128, 512], F32, tag="wk")
                for o in (0, 256):
                    nc.tensor.matmul(
                        qp[:, o:o + 256], w_qu8_sb[:, :, 128 * j:128 * j + 128],
                        q8[:, :, o:o + 256],
                        start=True, stop=True, perf_mode=DR)
                evac(on_act, QT8[:, j // 2, j % 2, off:off + 512], qp[:],
                     b_qu_sb[:, j:j + 1])

            def unit_V(pool, p, q, kvs):
                k = 4 * p + q
                vp = pool.tile([128, 512], F32, tag="wk")
                nc.tensor.matmul(vp[:], kvs[:, 128 * q:128 * q + 128],
                                 w_kvu_v_sb[:], start=True, stop=True)
                nc.vector.tensor_tensor(
                    V[:, k, :, 0:64],
                    vp[:].rearrange("p (h c) -> p h c", c=64),
                    bvb[:].rearrange("p (h c) -> p h c", c=64), ALU.add)

            def piece_units(pool, p, on_act=False, only=None):
                state = {}

                def mk_kv():
                    state["kvs"] = unit_kv(pool, p)

                def mk_q8():
                    q8 = latp.tile([128, 2, 512], FP8, tag="q8")
                    state["q8"] = q8
                    unit_q(pool, p, 0, q8)
                units = [("kv", mk_kv), ("q", mk_q8),
                         ("q", lambda: unit_q(pool, p, 1, state["q8"]))]
                units += [("KT", (lambda j=j: unit_KT(pool, p, j,
                                                      state["kvs"])))
                          for j in range(4)]
                units += [("QT", (lambda j=j: unit_QT(pool, p, j, state["q8"],
                                                      on_act)))
                          for j in range(4)]
                units += [("V", (lambda q=q: unit_V(pool, p, q, state["kvs"])))
                          for q in range(4)]
                for kind, u in units:
                    if only is None or kind in only:
                        yield u

            def unit_E(pool, si, o, dma_act=False):
                op = pool.tile([128, 512], F32, tag="wk")
                for cc in range(4):
                    nc.tensor.matmul(
                        op[:], ctxT[:, cc, 128 * si:128 * si + 128],
                        w_o_sb[:, cc, 512 * o:512 * o + 512],
                        start=(cc == 0), stop=(cc == 3))
                ob = obp.tile([128, 512], F32, tag="ob")
                nc.vector.tensor_copy(ob[:], op[:])
                eng = nc.scalar if dma_act else nc.sync
                eng.dma_start(
                    out=out_d[128 * si:128 * si + 128, 512 * o:512 * o + 512],
                    in_=ob[:])

            def unit_E1(pool, par, si, o):
                """Out-proj partial over cc 0,1 -> bf16 staging tile."""
                op = pool.tile([128, 512], F32, tag="wk")
                for cc in range(2):
                    nc.tensor.matmul(
                        op[:], ctxT[:, cc, 128 * si:128 * si + 128],
                        w_o_sb[:, cc, 512 * o:512 * o + 512],
                        start=(cc == 0), stop=(cc == 1))
                dst = par[:, 2 * (si - NT // 2) + o, :]
                nc.vector.tensor_copy(dst, op[:])

            def unit_E2(pool, par, si, o, alt=False):
                """cc2+cc3 matmuls, staged partial re-injected via identity
                matmul; plain-copy evac + DMA alternate between engines."""
                op = pool.tile([128, 512], F32, tag="wk")
                for cc in (2, 3):
                    nc.tensor.matmul(
                        op[:], ctxT[:, cc, 128 * si:128 * si + 128],
                        w_o_sb[:, cc, 512 * o:512 * o + 512],
                        start=(cc == 2), stop=False)
                nc.tensor.matmul(
                    op[:], ident16[:], par[:, 2 * (si - NT // 2) + o, :],
                    start=False, stop=True)
                ob = obp.tile([128, 512], F32, tag="ob")
                if alt:
                    nc.scalar.activation(ob[:], op[:], AF.Identity, bias=0.0)
                else:
                    nc.vector.tensor_copy(ob[:], op[:])
                eng = nc.scalar if alt else nc.sync
                eng.dma_start(
                    out=out_d[128 * si:128 * si + 128, 512 * o:512 * o + 512],
                    in_=ob[:])

            class Filler:
                """Dispenses queued work units evenly over `slots` calls."""
                def __init__(self, units, slots):
                    self.units = list(units)
                    self.slots = max(1, slots)
                    self.acc = 0.0
                    self.rate = len(self.units) / self.slots

                def __call__(self):
                    self.acc += self.rate
                    while self.acc >= 1.0 and self.units:
                        self.units.pop(0)()
                        self.acc -= 1.0

                def drain(self):
                    while self.units:
                        self.units.pop(0)()

            # ---- pieces 0..NP/2-1: deep scoped PSUM pool, ACT evacs -------
            with tc.tile_pool(name="pwk0", bufs=4, space="PSUM") as pwk0:
                # warm-up matmuls ramp the PE p-state while DMAs land
                warm = pwk0.tile([128, 512], F32, tag="warm")
                for i in range(24):
                    nc.tensor.matmul(warm[:, 0:128], ident16[:],
                                     ident16[:, 0:128],
                                     start=(i == 0), stop=(i == 23))
                for p in range(NP // 2):
                    for u in piece_units(pwk0, p, on_act=True):
                        u()


            def attn_phase(j, heads, filler, psc, pctx):
                """Attention for s-half j over `heads`, emitted with QK one
                chunk ahead of PV so exp never waits at head boundaries."""
                s0 = SH * j
                kmax = (SH // 128) * (j + 1)
                nbank = SH // 512
                last_k = {
                    bi: min(kmax - 1, (s0 + 512 * (bi + 1)) // 128 - 1)
                    for bi in range(nbank)
                }
                # chunk packing: complementary fd values share one score
                # tile (and one exp instruction) to amortize the ACT access
                # bubble.  Packs are emitted in order; PSUM accumulation
                # start/stop flags follow emission order per bank.
                if kmax == 8:
                    packs = [[0], [1, 7], [2, 6], [3, 5], [4]]
                else:
                    packs = [[k] for k in range(9)] + [[9, 15], [10, 14],
                                                       [11, 13], [12]]
                # emission order of chunks, for per-bank start/stop flags
                order = [k for pk in packs for k in pk]

                def chunk_geom(k):
                    t0 = 128 * k
                    ss = max(s0, t0)
                    return t0, ss, s0 + SH - ss, ss - s0

                bank_first, bank_last = {}, {}
                for pos, k in enumerate(order):
                    _, _, fd, rel = chunk_geom(k)
                    for bi in range(nbank):
                        a2 = max(rel, 512 * bi)
                        b2 = min(SH, 512 * bi + 512)
                        if a2 >= b2:
                            continue
                        if bi not in bank_first:
                            bank_first[bi] = k
                        bank_last[bi] = k

                recs = []
                for h in heads:
                    g, a = h // 4, h % 4
                    hst = {}
                    for pi, pk in enumerate(packs):
                        rec = {}
                        geo = [chunk_geom(k) for k in pk]
                        cos = []
                        co = 0
                        for (_, _, fd, _) in geo:
                            cos.append(co)
                            co += fd

                        def qk(rec=rec, g=g, a=a, pk=pk, geo=geo, cos=cos):
                            sc = psc.tile([128, SH], F32, tag="sc")
                            rec["sc"] = sc  # noqa
                            for k, (t0, ss, fd, rel), co in zip(pk, geo, cos):
                                for o2, w2 in _pieces(fd, 256):
                                    nc.tensor.matmul(
                                        sc[:, co + o2:co + o2 + w2],
                                        KT8[32 * a:32 * a + 32, g, :,
                                            t0:t0 + 128],
                                        QT8[32 * a:32 * a + 32, g, :,
                                            ss + o2:ss + o2 + w2],
                                        start=True, stop=True, perf_mode=DR,
                                        tile_position=(32 * a, 0))

                        def expaff(rec=rec, pk=pk, geo=geo, cos=cos):
                            ex = exb.tile([128, SH], BF16, tag="ex")
                            rec["ex"] = ex  # noqa
                            w = cos[-1] + geo[-1][2]
                            nc.scalar.activation(ex[:, :w], rec["sc"][:, :w],
                                                 AF.Exp, scale=0.125)
                            for k, (t0, ss, fd, rel), co in zip(pk, geo, cos):
                                if t0 >= s0:
                                    nc.gpsimd.affine_select(
                                        out=ex[:, co:co + 128],
                                        in_=ex[:, co:co + 128],
                                        pattern=[[1, 128]],
                                        compare_op=ALU.is_ge,
                                        fill=0.0, base=0,
                                        channel_multiplier=-1)

                        def pv(rec=rec, hst=hst, h=h, pk=pk, geo=geo, cos=cos):
                            if pk[0] == 0:
                                ctx = pctx.tile([65, SH], F32, tag="ctx")
                                hst["ctx"] = ctx
                            for k, (t0, ss, fd, rel), co in zip(pk, geo, cos):
                                for bi in range(nbank):
                                    a2 = max(rel, 512 * bi)
                                    b2 = min(SH, 512 * bi + 512)
                                    if a2 >= b2:
                                        continue
                                    nc.tensor.matmul(
                                        hst["ctx"][:, a2:b2], V[:, k, h, :],
                                        rec["ex"][:, co + a2 - rel:
                                                  co + b2 - rel],
                                        start=(bank_first[bi] == k),
                                        stop=(bank_last[bi] == k))

                        rec.update(qk=qk, expaff=expaff, pv=pv)
                        if pi == len(packs) - 1:
                            def norm(hst=hst, h=h):
                                # copy ctx out + reciprocal on DVE (releases
                                # the PSUM ctx buffer early); broadcast and
                                # multiply ride the lightly-loaded Pool engine
                                ctx = hst["ctx"]
                                raw = nrm.tile([64, SH], F32, tag="raw")
                                nc.vector.tensor_copy(raw[:], ctx[0:64, :])
                                rc = nrm.tile([1, SH], F32, tag="rec")
                                nc.vector.reciprocal(rc[:], ctx[64:65, :])
                                rbc = nrm.tile([64, SH], F32, tag="rbc")
                                nc.gpsimd.partition_broadcast(rbc[:],
                                                              rc[0:1, :])
                                po = 64 * (h % 2)
                                nc.gpsimd.tensor_tensor(
                                    ctxT[po:po + 64, h // 2, s0:s0 + SH],
                                    raw[:], rbc[:], ALU.mult)
                            rec["norm"] = norm
                        recs.append(rec)
                recs[0]["qk"]()
                for i, rec in enumerate(recs):
                    rec["expaff"]()
                    if i + 1 < len(recs):
                        recs[i + 1]["qk"]()
                    rec["pv"]()
                    if "norm" in rec:
                        rec["norm"]()
                    filler()

            # ---- attention (+ pieces 2-3 and first-half out-proj fillers) -
            with (
                tc.tile_pool(name="psc", bufs=2, space="PSUM") as psc,
                tc.tile_pool(name="pctx", bufs=1, space="PSUM") as pctx,
                tc.tile_pool(name="pwk", bufs=2, space="PSUM") as pwk,
            ):
                p2, p3 = NP // 2, NP // 2 + 1
                units_j0 = []
                st = {}
                for p in (p2, p3):
                    def mk_kv(p=p):
                        st[f"kvs{p}"] = unit_kv(pwk, p)
                    def mk_q8(p=p):
                        q8 = latp.tile([128, 2, 512], FP8, tag="q8")
                        st[f"q8{p}"] = q8
                        unit_q(pwk, p, 0, q8)
                    units_j0.append(mk_kv)
                    units_j0.append(mk_q8)
                    units_j0.append(lambda p=p: unit_q(pwk, p, 1, st[f"q8{p}"]))
                units_j0 += [(lambda p=p, j=j: unit_QT(pwk, p, j, st[f"q8{p}"]))
                             for p in (p2, p3) for j in range(4)]
                f0 = Filler(units_j0, 48)
                attn_phase(0, range(NHL), f0, psc, pctx)
                f0.drain()

                # j1 fillers: burst of late-consumed evacs (first consumer is
                # chunk >= 8 of head 0), then first-half out-proj
                burst = [(lambda p=p, j=j: unit_KT(pwk, p, j, st[f"kvs{p}"]))
                         for p in (p2, p3) for j in range(4)]
                burst += [(lambda p=p, q=q: unit_V(pwk, p, q, st[f"kvs{p}"]))
                          for p in (p2, p3) for q in range(4)]
                units_E = [
                    (lambda si=si, o=o: unit_E(pwk, si, o))
                    for si in range(NT // 2) for o in range(2)
                ]
                par = wts.tile([128, NT, 512], BF16, name="par")
                units_E1 = [
                    (lambda si=si, o=o: unit_E1(pwk, par, si, o))
                    for si in range(NT // 2, NT) for o in range(2)
                ]
                pk_per_head = 13         # packs per head in j1
                fb = Filler(burst, 7)
                fe = Filler(units_E, 4 * pk_per_head - 7)
                f1e = Filler(units_E1, 4 * pk_per_head)
                slot = [0]

                def f1():
                    slot[0] += 1
                    if fb.units:
                        fb()
                    elif fe.units:
                        fe()
                    elif slot[0] > 4 * pk_per_head:
                        f1e()
                attn_phase(1, range(NHL), f1, psc, pctx)
                fb.drain()
                fe.drain()
                f1e.drain()

            # ---- tail: remaining out-proj with a deep pool, ACT evacs -----
            with tc.tile_pool(name="ptl", bufs=4, space="PSUM") as ptl:
                for i, si in enumerate(range(NT // 2, NT)):
                    for o in range(2):
                        unit_E2(ptl, par, si, o, alt=(o == 1))

    nc.finalize()
    return nc


def _perm512():
    """Column permutation for w_kvu_k / w_qu so that PSUM chunk j, row
    32a+p corresponds to head 4*(j//2)+a, dim 32*(j%2)+p."""
    perm = np.empty(512, dtype=np.int64)
    for j in range(4):
        for a in range(4):
            for p in range(32):
                perm[128 * j + 32 * a + p] = 64 * (4 * (j // 2) + a) + 32 * (j % 2) + p
    return perm


def shard_inputs(inputs, S=2048):
    """Build the 8 per-core input maps from full inputs (host-side prep)."""
    f32 = lambda a: np.ascontiguousarray(np.asarray(a, dtype=np.float32))
    bf = lambda a: np.ascontiguousarray(
        np.asarray(a, dtype=np.float32).astype(ml_dtypes.bfloat16))
    fp8 = lambda a: np.ascontiguousarray(
        np.asarray(a, dtype=np.float32).astype(ml_dtypes.float8_e4m3))
    x = f32(inputs["x"])
    w_kvc, b_kvc = f32(inputs["w_kvc"]), f32(inputs["b_kvc"])
    w_kvu, b_kvu = f32(inputs["w_kvu"]), f32(inputs["b_kvu"])
    w_qc, b_qc = f32(inputs["w_qc"]), f32(inputs["b_qc"])
    w_qu, b_qu = f32(inputs["w_qu"]), f32(inputs["b_qu"])
    w_o, b_o = f32(inputs["w_o"]), f32(inputs["b_o"])
    perm = _perm512()
    in_maps = []
    for core in range(NCORES):
        b = core // 2
        g2 = core % 2
        ks = slice(512 * g2, 512 * g2 + 512)            # K-feature slice
        vs = slice(DIM + 512 * g2, DIM + 512 * g2 + 512)  # V-feature slice
        in_maps.append({
            "x": bf(x[b].T.reshape(ND, 128, S).transpose(1, 0, 2)),
            "x8": fp8(x[b].T.reshape(ND, 2, 64, S).transpose(2, 0, 1, 3)),
            "w_kvc": bf(w_kvc.reshape(ND, 128, LAT).transpose(1, 0, 2)),
            "w_qc8": fp8(w_qc.reshape(ND, 2, 64, QR).transpose(2, 0, 1, 3)),
            "w_kvu_k": bf(w_kvu[:, ks][:, perm]),
            "w_qu8": fp8(w_qu[:, ks][:, perm].reshape(2, 128, 512).transpose(1, 0, 2)),
            "w_kvu_v": bf(w_kvu[:, vs]),
            "w_o": bf(w_o[ks, :].reshape(4, 128, DIM).transpose(1, 0, 2)),
            "b_kvc": f32(b_kvc.reshape(LAT, 1)),
            "b_qc": f32(b_qc.reshape(2, 128).T),
            "b_qu": f32(b_qu[ks][perm].reshape(4, 128).T),
            "b_kvu_k": f32(b_kvu[ks][perm].reshape(4, 128).T),
            "b_kvu_v": f32(b_kvu[vs].reshape(1, 512)),
        })
    return in_maps


def kernel(**inputs) -> np.ndarray:
    from concourse.bass_utils import run_bass_kernel_spmd

    x = np.asarray(inputs["x"])
    S = x.shape[1]
    nc = build_mla(S=S)
    in_maps = shard_inputs(inputs, S=S)
    res = run_bass_kernel_spmd(nc, in_maps, list(range(NCORES))).results
    b_o = np.asarray(inputs["b_o"], dtype=np.float32)
    out = np.empty((B, S, DIM), dtype=np.float32)
    for b in range(B):
        out[b] = res[2 * b]["out"] + res[2 * b + 1]["out"] + b_o
    return out
